# revision 1
# baseline (speedup 1.0000x reference)
"""Multi-head attention forward on 8 Trainium2 NeuronCores.

Sharding: batch (2) x head-groups (4 heads each) -> 8 cores, Megatron-style.
Each core computes q/k/v projections for its 256-dim head slice, attention
for its 4 heads, and a partial output projection; the host sums the 4
partials per batch element and adds the output bias.

Device-side layout choices (all picked to avoid fp32 transposes on chip):
 - host passes x^T (embed-major) activations, so the projection matmuls
   contract embed on partitions directly
 - q and k are produced head-transposed [d, s]; the scores matmul
   (lhsT=k^T chunk, rhs=q^T) then emits scores^T [k_seq, q_seq] whose
   partition dim is k_seq -- exactly what the ctx matmul needs to contract
 - softmax skips max-subtraction (scores ~ N(0,1), |s| < ~6 => exp is safe
   in fp32); the denominator Z rides along as a fused ones-column of v in
   the ctx matmul (lhsT = [v_h | 1], M=65)
 - normalization by 1/Z commutes past nothing (per-head Z), so ctx^T is
   scaled via gpsimd partition_broadcast of the reciprocal row
"""

import numpy as np

import concourse.bass as bass
import concourse.tile as tile
from concourse import bacc, mybir
from concourse.bass_utils import run_bass_kernel_spmd

F32 = mybir.dt.float32

B = 2
S = 2048
E = 1024
H = 16
D = 64
HPC = 4          # heads per core
EC = HPC * D     # 256: embed slice per core
NCORES = 8
KO = E // 128    # 8 contraction chunks for the projections


def build_mha(tc: tile.TileContext, S_=S):
    nc = tc.nc
    SI = S_ // 512       # 512-wide seq chunks
    JC = S_ // 128       # 128-wide key chunks
    NH = S_ // 1024 if S_ >= 1024 else 1   # i-halves
    IW = min(S_, 1024)   # i-block width
    IIW = IW // 512      # 512-wide slices per i-block

    xq = nc.dram_tensor("xq", [E, S_], F32, kind="ExternalInput").ap()
    xk = nc.dram_tensor("xk", [E, S_], F32, kind="ExternalInput").ap()
    xv = nc.dram_tensor("xv", [E, S_], F32, kind="ExternalInput").ap()
    wq = nc.dram_tensor("wq", [E, EC], F32, kind="ExternalInput").ap()
    wk = nc.dram_tensor("wk", [E, EC], F32, kind="ExternalInput").ap()
    wv = nc.dram_tensor("wv", [E, EC], F32, kind="ExternalInput").ap()
    wo = nc.dram_tensor("wo", [EC, E], F32, kind="ExternalInput").ap()
    bq = nc.dram_tensor("bq", [EC], F32, kind="ExternalInput").ap()
    bk = nc.dram_tensor("bk", [EC], F32, kind="ExternalInput").ap()
    bv = nc.dram_tensor("bv", [EC], F32, kind="ExternalInput").ap()
    out = nc.dram_tensor("out", [S_, E], F32, kind="ExternalOutput").ap()

    xq3 = xq.rearrange("(ko p) s -> p ko s", p=128)
    xk3 = xk.rearrange("(ko p) s -> p ko s", p=128)
    xv3 = xv.rearrange("(ko p) s -> p ko s", p=128)

    with (
        tc.tile_pool(name="wpool", bufs=1) as wpool,
        tc.tile_pool(name="persist", bufs=1) as persist,
        tc.tile_pool(name="xin", bufs=3) as xin,
        tc.tile_pool(name="expp", bufs=3) as expp,
        tc.tile_pool(name="csbp", bufs=2) as csbp,
        tc.tile_pool(name="rzp", bufs=2) as rzp,
        tc.tile_pool(name="rzbp", bufs=2) as rzbp,
        tc.tile_pool(name="outp", bufs=3) as outp,
        tc.tile_pool(name="psA", bufs=2, space="PSUM") as psA,
        tc.tile_pool(name="psS", bufs=2, space="PSUM") as psS,
        tc.tile_pool(name="psC", bufs=1, space="PSUM") as psC,
    ):
        # ---- weights / biases to SBUF ----
        wq_sb = wpool.tile([128, KO, EC], F32)
        wk_sb = wpool.tile([128, KO, EC], F32)
        wv_sb = wpool.tile([128, KO, EC], F32)
        wo_sb = wpool.tile([128, 2, E], F32)
        nc.sync.dma_start(wq_sb[:], wq.rearrange("(ko p) m -> p ko m", p=128))
        nc.sync.dma_start(wk_sb[:], wk.rearrange("(ko p) m -> p ko m", p=128))
        nc.sync.dma_start(wv_sb[:], wv.rearrange("(ko p) m -> p ko m", p=128))
        nc.sync.dma_start(wo_sb[:], wo.rearrange("(kf p) e -> p kf e", p=128))
        bq_sb = wpool.tile([128, 2], F32)
        bk_sb = wpool.tile([128, 2], F32)
        nc.sync.dma_start(bq_sb[:], bq.rearrange("(c p) -> p c", p=128))
        nc.sync.dma_start(bk_sb[:], bk.rearrange("(c p) -> p c", p=128))
        bv_row = wpool.tile([1, EC], F32)
        nc.sync.dma_start(bv_row[:], bv[None, :])
        bv_bc = wpool.tile([128, EC], F32)
        nc.gpsimd.partition_broadcast(bv_bc[:], bv_row[:])

        # ---- persistent activation tiles ----
        qT = persist.tile([128, 2, S_], F32)   # [d(2 heads), head-pair, s]
        kT = persist.tile([128, 2, S_], F32)
        va = persist.tile([128, JC, HPC * 65], F32)  # [s%128, s//128, h*(64+1)]
        ctxn = persist.tile([128, 2, S_], F32)       # normalized ctx^T

        va4 = va.rearrange("p j (h t) -> p j h t", t=65)
        nc.vector.memset(va4[:, :, :, 64], 1.0)

        # ---- projections ----
        for si in range(SI):
            sl = bass.ts(si, 512)
            xq_t = xin.tile([128, KO, 512], F32, tag="xin")
            nc.sync.dma_start(xq_t[:], xq3[:, :, sl])
            for c in range(2):
                pq = psA.tile([128, 512], F32, tag="ps_a")
                for ko in range(KO):
                    nc.tensor.matmul(pq[:], wq_sb[:, ko, bass.ts(c, 128)],
                                     xq_t[:, ko, :],
                                     start=(ko == 0), stop=(ko == KO - 1))
                nc.vector.tensor_scalar_add(qT[:, c, sl], pq[:], bq_sb[:, c:c + 1])
        for si in range(SI):
            sl = bass.ts(si, 512)
            xk_t = xin.tile([128, KO, 512], F32, tag="xin")
            nc.sync.dma_start(xk_t[:], xk3[:, :, sl])
            for c in range(2):
                pk = psA.tile([128, 512], F32, tag="ps_a")
                for ko in range(KO):
                    nc.tensor.matmul(pk[:], wk_sb[:, ko, bass.ts(c, 128)],
                                     xk_t[:, ko, :],
                                     start=(ko == 0), stop=(ko == KO - 1))
                nc.vector.tensor_scalar_add(kT[:, c, sl], pk[:], bk_sb[:, c:c + 1])
        for si in range(SI):
            sl = bass.ts(si, 512)
            xv_t = xin.tile([128, KO, 512], F32, tag="xin")
            nc.sync.dma_start(xv_t[:], xv3[:, :, sl])
            for sj in range(4):
                jc = si * 4 + sj
                pv = psA.tile([128, 512], F32, tag="ps_a")
                for ko in range(KO):
                    nc.tensor.matmul(pv[:, 0:EC],
                                     xv_t[:, ko, bass.ts(sj, 128)],
                                     wv_sb[:, ko, :],
                                     start=(ko == 0), stop=(ko == KO - 1))
                for h in range(HPC):
                    nc.vector.tensor_add(va[:, jc, h * 65:h * 65 + 64],
                                         pv[:, bass.ts(h, 64)],
                                         bv_bc[:, bass.ts(h, 64)])

        # ---- attention + output projection ----
        for half in range(NH):
            for h in range(HPC):
                p0 = 64 * (h % 2)
                c = h // 2
                C = psC.tile([65, IW], F32, tag="ps_c")
                for jc in range(JC):
                    S_t = psS.tile([128, IW], F32, tag="ps_s")
                    for ii in range(IIW):
                        isl = bass.ds(half * IW + ii * 512, 512)
                        nc.tensor.matmul(S_t[:, bass.ts(ii, 512)],
                                         kT[p0:p0 + 64, c, bass.ts(jc, 128)],
                                         qT[p0:p0 + 64, c, isl],
                                         start=True, stop=True)
                    eT = expp.tile([128, IW], F32, tag="expp")
                    nc.scalar.activation(eT[:], S_t[:],
                                         mybir.ActivationFunctionType.Exp)
                    for ii in range(IIW):
                        nc.tensor.matmul(C[:, bass.ts(ii, 512)],
                                         va[:, jc, h * 65:h * 65 + 65],
                                         eT[:, bass.ts(ii, 512)],
                                         start=(jc == 0), stop=(jc == JC - 1))
                # normalize: ctxn = C[0:64] / C[64]
                csb = csbp.tile([65, IW], F32, tag="csb")
                nc.vector.tensor_copy(csb[:], C[:])
                rz = rzp.tile([1, IW], F32, tag="rz")
                nc.vector.reciprocal(rz[0:1, :], csb[64:65, :])
                rzb = rzbp.tile([64, IW], F32, tag="rzb")
                nc.gpsimd.partition_broadcast(rzb[:], rz[:])
                nc.vector.tensor_tensor(ctxn[p0:p0 + 64, c,
                                              bass.ds(half * IW, IW)],
                                        csb[0:64, :], rzb[:],
                                        mybir.AluOpType.mult)
            # output projection for this i-half
            for sc in range(IW // 128):
                s0 = half * IW + sc * 128
                for eo in range(2):
                    po = psA.tile([128, 512], F32, tag="ps_a")
                    for kf in range(2):
                        nc.tensor.matmul(po[:],
                                         ctxn[:, kf, bass.ds(s0, 128)],
                                         wo_sb[:, kf, bass.ts(eo, 512)],
                                         start=(kf == 0), stop=(kf == 1))
                    ot = outp.tile([128, 512], F32, tag="ot")
                    nc.vector.tensor_copy(ot[:], po[:])
                    nc.sync.dma_start(out[bass.ds(s0, 128), bass.ts(eo, 512)],
                                      ot[:])


_CACHED = {}


def _get_nc(S_=S):
    if S_ not in _CACHED:
        nc = bacc.Bacc("TRN2", target_bir_lowering=False, debug=False)
        with tile.TileContext(nc) as tc:
            build_mha(tc, S_)
        nc.compile()
        _CACHED[S_] = nc
    return _CACHED[S_]


def shard_inputs(query, key, value, Wq, bq, Wk, bk, Wv, bv, Wo, bo):
    """Build the 8 per-core input maps (numpy, fp32)."""
    scale = np.float32(1.0 / np.sqrt(D))
    in_maps = []
    for core in range(NCORES):
        b = core // HPC
        g = core % HPC
        hs = slice(g * EC, (g + 1) * EC)
        in_maps.append({
            "xq": np.ascontiguousarray(query[b].T, np.float32),
            "xk": np.ascontiguousarray(key[b].T, np.float32),
            "xv": np.ascontiguousarray(value[b].T, np.float32),
            "wq": np.ascontiguousarray(Wq[hs, :].T, np.float32),
            "wk": np.ascontiguousarray(Wk[hs, :].T * scale, np.float32),
            "wv": np.ascontiguousarray(Wv[hs, :].T, np.float32),
            "wo": np.ascontiguousarray(Wo[:, hs].T, np.float32),
            "bq": np.ascontiguousarray(bq[hs], np.float32),
            "bk": np.ascontiguousarray(bk[hs] * scale, np.float32),
            "bv": np.ascontiguousarray(bv[hs], np.float32),
        })
    return in_maps


def combine_outputs(results, bo):
    out = np.zeros((B, S, E), np.float32)
    for core in range(NCORES):
        out[core // HPC] += results[core]["out"]
    out += np.asarray(bo, np.float32)[None, None, :]
    return out


def kernel(query, key, value, Wq, bq, Wk, bk, Wv, bv, Wo, bo):
    nc = _get_nc()
    in_maps = shard_inputs(query, key, value, Wq, bq, Wk, bk, Wv, bv, Wo, bo)
    res = run_bass_kernel_spmd(nc, in_maps, list(range(NCORES)))
    return combine_outputs(res.results, bo)


# revision 4
# speedup vs baseline: 47.4938x; 47.4938x over previous
"""Multi-head attention forward on 8 Trainium2 NeuronCores.

Sharding: batch (2) x head-groups (4 heads each) -> 8 cores, Megatron-style.
Each core computes q/k/v projections for its 256-dim head slice, attention
for its 4 heads, and a partial output projection; the host sums the 4
partials per batch element and adds the output bias.

Device-side layout choices (all picked to avoid fp32 transposes on chip):
 - host passes x^T (embed-major) activations, so the projection matmuls
   contract embed on partitions directly
 - q and k are produced head-transposed [d, s]; the scores matmul
   (lhsT=k^T chunk, rhs=q^T) then emits scores^T [k_seq, q_seq] whose
   partition dim is k_seq -- exactly what the ctx matmul needs to contract
 - softmax skips max-subtraction (scores ~ N(0,1), |s| < ~6 => exp is safe
   in fp32); the denominator Z rides along as a fused ones-column of v in
   the ctx matmul (lhsT = [v_h | 1], M=65)
 - normalization by 1/Z commutes past nothing (per-head Z), so ctx^T is
   scaled via gpsimd partition_broadcast of the reciprocal row
"""

import numpy as np

import concourse.bass as bass
import concourse.tile as tile
from concourse import bacc, mybir
from concourse.bass_utils import run_bass_kernel_spmd

F32 = mybir.dt.float32
F32R = mybir.dt.float32r

B = 2
S = 2048
E = 1024
H = 16
D = 64
HPC = 4          # heads per core
EC = HPC * D     # 256: embed slice per core
NCORES = 8
KO = E // 128    # 8 contraction chunks for the projections


def build_mha(tc: tile.TileContext, S_=S):
    nc = tc.nc
    SI = S_ // 512       # 512-wide seq chunks
    JC = S_ // 128       # 128-wide key chunks
    NH = S_ // 1024 if S_ >= 1024 else 1   # i-halves
    IW = min(S_, 1024)   # i-block width
    IIW = IW // 512      # 512-wide slices per i-block

    xq = nc.dram_tensor("xq", [E, S_], F32R, kind="ExternalInput").ap()
    xk = nc.dram_tensor("xk", [E, S_], F32R, kind="ExternalInput").ap()
    xv = nc.dram_tensor("xv", [E, S_], F32R, kind="ExternalInput").ap()
    wq = nc.dram_tensor("wq", [E, EC], F32R, kind="ExternalInput").ap()
    wk = nc.dram_tensor("wk", [E, EC], F32R, kind="ExternalInput").ap()
    wv = nc.dram_tensor("wv", [E, EC], F32R, kind="ExternalInput").ap()
    wo = nc.dram_tensor("wo", [EC, E], F32R, kind="ExternalInput").ap()
    bq = nc.dram_tensor("bq", [EC], F32, kind="ExternalInput").ap()
    bk = nc.dram_tensor("bk", [EC], F32, kind="ExternalInput").ap()
    bv = nc.dram_tensor("bv", [EC], F32, kind="ExternalInput").ap()
    out = nc.dram_tensor("out", [S_, E], F32, kind="ExternalOutput").ap()

    xq3 = xq.rearrange("(ko p) s -> p ko s", p=128)
    xk3 = xk.rearrange("(ko p) s -> p ko s", p=128)
    xv3 = xv.rearrange("(ko p) s -> p ko s", p=128)

    with (
        tc.tile_pool(name="wpool", bufs=1) as wpool,
        tc.tile_pool(name="persist", bufs=1) as persist,
        tc.tile_pool(name="xin", bufs=3) as xin,
        tc.tile_pool(name="expp", bufs=3) as expp,
        tc.tile_pool(name="csbp", bufs=2) as csbp,
        tc.tile_pool(name="rzp", bufs=2) as rzp,
        tc.tile_pool(name="rzbp", bufs=2) as rzbp,
        tc.tile_pool(name="outp", bufs=3) as outp,
        tc.tile_pool(name="psA", bufs=2, space="PSUM") as psA,
        tc.tile_pool(name="psS", bufs=2, space="PSUM") as psS,
        tc.tile_pool(name="psC", bufs=1, space="PSUM") as psC,
    ):
        # ---- weights / biases to SBUF ----
        wq_sb = wpool.tile([128, KO, EC], F32R)
        wk_sb = wpool.tile([128, KO, EC], F32R)
        wv_sb = wpool.tile([128, KO, EC], F32R)
        wo_sb = wpool.tile([128, 2, E], F32R)
        nc.sync.dma_start(wq_sb[:], wq.rearrange("(ko p) m -> p ko m", p=128))
        nc.sync.dma_start(wk_sb[:], wk.rearrange("(ko p) m -> p ko m", p=128))
        nc.sync.dma_start(wv_sb[:], wv.rearrange("(ko p) m -> p ko m", p=128))
        nc.sync.dma_start(wo_sb[:], wo.rearrange("(kf p) e -> p kf e", p=128))
        bq_sb = wpool.tile([128, 2], F32)
        bk_sb = wpool.tile([128, 2], F32)
        nc.sync.dma_start(bq_sb[:], bq.rearrange("(c p) -> p c", p=128))
        nc.sync.dma_start(bk_sb[:], bk.rearrange("(c p) -> p c", p=128))
        bv_row = wpool.tile([1, EC], F32)
        nc.sync.dma_start(bv_row[:], bv[None, :])
        bv_bc = wpool.tile([128, EC], F32)
        nc.gpsimd.partition_broadcast(bv_bc[:], bv_row[:])

        # ---- persistent activation tiles ----
        qT = persist.tile([128, 2, S_], F32R)   # [d(2 heads), head-pair, s]
        kT = persist.tile([128, 2, S_], F32R)
        va = persist.tile([128, JC, HPC * 65], F32R)  # [s%128, s//128, h*(64+1)]
        ctxn = persist.tile([128, 2, S_], F32R)       # normalized ctx^T

        va4 = va[:].bitcast(F32).rearrange("p j (h t) -> p j h t", t=65)
        nc.vector.memset(va4[:, :, :, 64], 1.0)

        # ---- projections ----
        for si in range(SI):
            sl = bass.ts(si, 512)
            xq_t = xin.tile([128, KO, 512], F32R, tag="xin")
            nc.sync.dma_start(xq_t[:], xq3[:, :, sl])
            for c in range(2):
                pq = psA.tile([128, 512], F32, tag="ps_a")
                for ko in range(KO):
                    nc.tensor.matmul(pq[:], wq_sb[:, ko, bass.ts(c, 128)],
                                     xq_t[:, ko, :],
                                     start=(ko == 0), stop=(ko == KO - 1))
                nc.vector.tensor_scalar_add(qT[:, c, sl], pq[:], bq_sb[:, c:c + 1])
        for si in range(SI):
            sl = bass.ts(si, 512)
            xk_t = xin.tile([128, KO, 512], F32R, tag="xin")
            nc.sync.dma_start(xk_t[:], xk3[:, :, sl])
            for c in range(2):
                pk = psA.tile([128, 512], F32, tag="ps_a")
                for ko in range(KO):
                    nc.tensor.matmul(pk[:], wk_sb[:, ko, bass.ts(c, 128)],
                                     xk_t[:, ko, :],
                                     start=(ko == 0), stop=(ko == KO - 1))
                nc.vector.tensor_scalar_add(kT[:, c, sl], pk[:], bk_sb[:, c:c + 1])
        for si in range(SI):
            sl = bass.ts(si, 512)
            xv_t = xin.tile([128, KO, 512], F32R, tag="xin")
            nc.sync.dma_start(xv_t[:], xv3[:, :, sl])
            for sj in range(4):
                jc = si * 4 + sj
                pv = psA.tile([128, 512], F32, tag="ps_a")
                for ko in range(KO):
                    nc.tensor.matmul(pv[:, 0:EC],
                                     xv_t[:, ko, bass.ts(sj, 128)],
                                     wv_sb[:, ko, :],
                                     start=(ko == 0), stop=(ko == KO - 1))
                for h in range(HPC):
                    nc.vector.tensor_add(va[:, jc, h * 65:h * 65 + 64],
                                         pv[:, bass.ts(h, 64)],
                                         bv_bc[:, bass.ts(h, 64)])

        # ---- attention + output projection ----
        for half in range(NH):
            for h in range(HPC):
                p0 = 64 * (h % 2)
                c = h // 2
                C = psC.tile([65, IW], F32, tag="ps_c")
                for jc in range(JC):
                    S_t = psS.tile([128, IW], F32, tag="ps_s")
                    for ii in range(IIW):
                        isl = bass.ds(half * IW + ii * 512, 512)
                        nc.tensor.matmul(S_t[:, bass.ts(ii, 512)],
                                         kT[p0:p0 + 64, c, bass.ts(jc, 128)],
                                         qT[p0:p0 + 64, c, isl],
                                         start=True, stop=True)
                    eT = expp.tile([128, IW], F32R, tag="expp")
                    nc.scalar.activation(eT[:], S_t[:],
                                         mybir.ActivationFunctionType.Exp)
                    for ii in range(IIW):
                        nc.tensor.matmul(C[:, bass.ts(ii, 512)],
                                         va[:, jc, h * 65:h * 65 + 65],
                                         eT[:, bass.ts(ii, 512)],
                                         start=(jc == 0), stop=(jc == JC - 1))
                # normalize: ctxn = C[0:64] / C[64]
                csb = csbp.tile([65, IW], F32, tag="csb")
                nc.vector.tensor_copy(csb[:], C[:])
                rz = rzp.tile([1, IW], F32, tag="rz")
                nc.vector.reciprocal(rz[0:1, :], csb[64:65, :])
                rzb = rzbp.tile([64, IW], F32, tag="rzb")
                nc.gpsimd.partition_broadcast(rzb[:], rz[:])
                nc.vector.tensor_tensor(ctxn[p0:p0 + 64, c,
                                              bass.ds(half * IW, IW)],
                                        csb[0:64, :], rzb[:],
                                        mybir.AluOpType.mult)
            # output projection for this i-half
            for sc in range(IW // 128):
                s0 = half * IW + sc * 128
                for eo in range(2):
                    po = psA.tile([128, 512], F32, tag="ps_a")
                    for kf in range(2):
                        nc.tensor.matmul(po[:],
                                         ctxn[:, kf, bass.ds(s0, 128)],
                                         wo_sb[:, kf, bass.ts(eo, 512)],
                                         start=(kf == 0), stop=(kf == 1))
                    ot = outp.tile([128, 512], F32, tag="ot")
                    nc.vector.tensor_copy(ot[:], po[:])
                    nc.sync.dma_start(out[bass.ds(s0, 128), bass.ts(eo, 512)],
                                      ot[:])


_CACHED = {}


def _get_nc(S_=S):
    if S_ not in _CACHED:
        nc = bacc.Bacc("TRN2", target_bir_lowering=False, debug=False)
        with tile.TileContext(nc) as tc:
            build_mha(tc, S_)
        nc.compile()
        _CACHED[S_] = nc
    return _CACHED[S_]


def shard_inputs(query, key, value, Wq, bq, Wk, bk, Wv, bv, Wo, bo):
    """Build the 8 per-core input maps (numpy, fp32)."""
    scale = np.float32(1.0 / np.sqrt(D))
    in_maps = []
    for core in range(NCORES):
        b = core // HPC
        g = core % HPC
        hs = slice(g * EC, (g + 1) * EC)
        in_maps.append({
            "xq": np.ascontiguousarray(query[b].T, np.float32),
            "xk": np.ascontiguousarray(key[b].T, np.float32),
            "xv": np.ascontiguousarray(value[b].T, np.float32),
            "wq": np.ascontiguousarray(Wq[hs, :].T, np.float32),
            "wk": np.ascontiguousarray(Wk[hs, :].T * scale, np.float32),
            "wv": np.ascontiguousarray(Wv[hs, :].T, np.float32),
            "wo": np.ascontiguousarray(Wo[:, hs].T, np.float32),
            "bq": np.ascontiguousarray(bq[hs], np.float32),
            "bk": np.ascontiguousarray(bk[hs] * scale, np.float32),
            "bv": np.ascontiguousarray(bv[hs], np.float32),
        })
    return in_maps


def combine_outputs(results, bo):
    out = np.zeros((B, S, E), np.float32)
    for core in range(NCORES):
        out[core // HPC] += results[core]["out"]
    out += np.asarray(bo, np.float32)[None, None, :]
    return out


def kernel(query, key, value, Wq, bq, Wk, bk, Wv, bv, Wo, bo):
    nc = _get_nc()
    in_maps = shard_inputs(query, key, value, Wq, bq, Wk, bk, Wv, bv, Wo, bo)
    res = run_bass_kernel_spmd(nc, in_maps, list(range(NCORES)))
    return combine_outputs(res.results, bo)
